# revision 41
# baseline (speedup 1.0000x reference)
# Trainium2 Bass kernel for nn_Binning (KPN denoiser-style net).
#
# Strategy: 8-way shard = batch(2) x 4 horizontal bands of the output.
# Each core computes a 34-row band of the final 136x136 output from a
# 64-row slice of the (row-padded) input, fully replicating the small conv
# weights. All convs run as 9 shifted matmuls accumulating in PSUM
# (channels on partitions, flattened rows*cols on the free axis, full
# 160-wide rows with wrap-around garbage columns that are provably never
# read for valid outputs). The recurrent SAME convs rely on zeroed
# cols >=146 and per-core row masks so band-local VALID convs reproduce
# global zero padding semantics.
import os
import sys

import numpy as np

for _p in ("/opt/trn_rl_repo",):
    if os.path.isdir(_p) and _p not in sys.path:
        sys.path.insert(0, _p)

import ml_dtypes

import concourse.bass as bass
import concourse.mybir as mybir
import concourse.tile as tile
from concourse.vector_clock import ScopedClock

W = 160
OB = 34          # output rows per band
NB = 4           # bands per batch element
LEAKY = 0.01
F32 = mybir.dt.float32
MM_DT = mybir.dt.bfloat16
MM_NP = ml_dtypes.bfloat16
AF = mybir.ActivationFunctionType
OP = mybir.AluOpType
_DISABLE_SAME = False  # debug: drop the -1 column shift (wrong numerics)


class _TC(tile.TileContext):
    """TileContext that spreads multi-semaphore waits across single-wait
    same-engine NOPs (this walrus build allows only one sync-wait per
    instruction)."""

    def _split_multi_waits(self):
        nc = self.nc
        cnt = [0]
        for fn in nc.m.functions:
            for blk in fn.blocks:
                insts = blk.instructions
                need = any(
                    i.sync_info is not None and i.sync_info.on_wait
                    and len(i.sync_info.on_wait) > 1 for i in insts)
                if not need:
                    continue
                new = []
                for i in insts:
                    si = i.sync_info
                    if si is not None and si.on_wait and len(si.on_wait) > 1:
                        waits = list(si.on_wait)
                        for w in waits[:-1]:
                            cnt[0] += 1
                            nop = mybir.InstNoOp(
                                name=f"I-waitsplit-{cnt[0]}", ins=[], outs=[])
                            nop.engine = i.engine
                            nop.sync_info = mybir.SyncInfo(
                                on_wait=[w], on_update=[])
                            nc.register_instruction(nop, overwrite=True)
                            new.append(nop)
                        si.on_wait = [waits[-1]]
                    new.append(i)
                while len(blk.instructions):
                    blk.instructions.pop()
                for i in new:
                    blk.instructions.append(i)

    def _drain_and_barrier(self, tick_clock, wait_clock):
        nc = self.nc
        self._split_multi_waits()
        drain_inst = nc.sync.drain()
        wait_clock.add_sem_waits(
            drain_inst.ins, ScopedClock({None: tick_clock.global_clock})
        )
        si = drain_inst.ins.sync_info
        waits = list(si.on_wait or [])
        if len(waits) > 1:
            si.on_wait = [waits[-1]]
            bb = nc.cur_bb.bb
            insts = bb.instructions
            assert insts[-1] is drain_inst.ins
            insts.pop()
            for w in waits[:-1]:
                n = nc.sync.nop(nofuse=True, hint="drain_wait_split")
                n.ins.sync_info = mybir.SyncInfo(on_wait=[w], on_update=[])
            insts.append(drain_inst.ins)
        nc.all_engine_barrier()
        popped = nc._tile_sem_poison_stack.pop()
        assert popped is self._sem_poison
        nc.clear_and_free_semaphores(list(self.sems.allocated().values()))
        nc.all_engine_barrier()


def _emit_conv(nc, psum_pool, ins, w_sb, n_kg, cout, h_out, out_base, out_row0,
               ncols, bias_ap, act):
    """One 3x3 conv layer: channels on partitions, 9 shifted matmuls
    accumulated in PSUM, 3 output rows (480 elems) per PSUM bank, 8 banks
    per round. `ins` = per-k-group flat [Cin, >= (h_in+1)*W] SBUF APs.
    For SAME-column convs the caller passes `ins` with its base already
    shifted by -1 element into the zeroed lead row. Evacuation: ACT
    Lrelu(psum+bias) or DVE psum+bias -> out_base[:, out_row0+r:, 0:ncols]."""
    r0 = 0
    while r0 < h_out:
        rem = h_out - r0
        nb = min(8, (rem + 2) // 3)
        banks = []
        for b in range(nb):
            rows_b = min(3, rem - 3 * b)
            if rows_b <= 0:
                break
            banks.append((b, rows_b))
        psums = [psum_pool.tile([cout, 3, W], F32, tag="ps", name=f"ps{r0}_{i}")
                 for i in range(len(banks))]
        for kg in range(n_kg):
            for t9 in range(9):
                dy, dx = divmod(t9, 3)
                lhsT = w_sb[:, kg * 9 + t9, :]
                start = kg == 0 and t9 == 0
                stop = kg == n_kg - 1 and t9 == 8
                for (b, rows_b), ps in zip(banks, psums):
                    off = (r0 + 3 * b + dy) * W + dx
                    n = rows_b * W
                    ps_flat = ps.rearrange("c h w -> c (h w)")
                    nc.tensor.matmul(
                        ps_flat[:, :n], lhsT, ins[kg][:, off:off + n],
                        start=start, stop=stop,
                    )
        for (b, rows_b), ps in zip(banks, psums):
            r = out_row0 + r0 + 3 * b
            o = out_base[:, r:r + rows_b, 0:ncols]
            if act == "lrelu":
                nc.scalar.activation(
                    o, ps[:, :rows_b, 0:ncols], AF.Lrelu,
                    bias=bias_ap, scale=1.0, alpha=LEAKY,
                )
            else:
                nc.vector.tensor_scalar(
                    o, ps[:, :rows_b, 0:ncols], bias_ap, None, OP.add
                )
        r0 += 3 * len(banks)


def _flat(ap):
    return ap.rearrange("c h w -> c (h w)")


def _same_in(ap, lead_rows=1):
    """Flat view starting one element before data row `lead_rows` (reads
    the zeroed lead row's last column for the SAME conv's column -1)."""
    sh = 0 if _DISABLE_SAME else 1
    return _flat(ap)[:, lead_rows * W - sh:]


def build_program(debug_stage=None):
    nc = bass.Bass("TRN2")

    # --- DRAM I/O ---
    x_d = nc.dram_tensor("x", (4, 67, 64, W), MM_DT, kind="ExternalInput")
    rad12_d = nc.dram_tensor("rad12", (4, 12, 38, 140), F32, kind="ExternalInput")
    masks_d = nc.dram_tensor("masks", (128, 6), MM_DT, kind="ExternalInput")
    bias_d = nc.dram_tensor("biases", (128, 14), F32, kind="ExternalInput")
    wd = {}
    wd["e1"] = nc.dram_tensor("w_e1", (67, 9, 128), MM_DT, kind="ExternalInput")
    for i in range(2, 8):
        wd[f"e{i}"] = nc.dram_tensor(f"w_e{i}", (128, 9, 128), MM_DT, kind="ExternalInput")
    wd["c1"] = nc.dram_tensor("w_c1", (128, 9, 128), MM_DT, kind="ExternalInput")
    wd["c2"] = nc.dram_tensor("w_c2", (128, 9, 128), MM_DT, kind="ExternalInput")
    wd["c3"] = nc.dram_tensor("w_c3", (128, 9, 12), MM_DT, kind="ExternalInput")
    wd["r1"] = nc.dram_tensor("w_r1", (128, 18, 128), MM_DT, kind="ExternalInput")
    wd["r2"] = nc.dram_tensor("w_r2", (128, 9, 128), MM_DT, kind="ExternalInput")
    wd["k1"] = nc.dram_tensor("w_k1", (128, 9, 128), MM_DT, kind="ExternalInput")
    wd["k2"] = nc.dram_tensor("w_k2", (128, 9, 100), MM_DT, kind="ExternalInput")
    sel_d = nc.dram_tensor("sel3", (120, 3), F32, kind="ExternalInput")
    out_d = nc.dram_tensor("out", (3, OB, 136), F32, kind="ExternalOutput")

    from contextlib import ExitStack
    with _TC(nc) as tc, ExitStack() as ctx:
        wpool = ctx.enter_context(tc.tile_pool(name="w", bufs=1))
        apool = ctx.enter_context(tc.tile_pool(name="a", bufs=1))
        epool = ctx.enter_context(tc.tile_pool(name="e", bufs=2))
        psum = ctx.enter_context(tc.tile_pool(name="p", bufs=8, space="PSUM"))

        # --- load constants ---
        ws = {}
        for key, d in wd.items():
            t = wpool.tile(list(d.shape), MM_DT, tag=f"w_{key}")
            nc.sync.dma_start(t[:], d[:])
            ws[key] = t
        bias_sb = wpool.tile([128, 14], F32, tag="bias")
        nc.sync.dma_start(bias_sb[:], bias_d[:])
        masks_sb = wpool.tile([128, 6], MM_DT, tag="masks")
        nc.sync.dma_start(masks_sb[:], masks_d[:])
        sel_sb = wpool.tile([120, 3], F32, tag="sel")
        nc.sync.dma_start(sel_sb[:], sel_d[:])

        def bias_ap(col, cout):
            return bias_sb[0:cout, col:col + 1]

        # stage tile: f32 scratch for the per-sp softmax/w_rad stage.
        # Compute-engine APs may only start at partitions 0/32/64, so the
        # layout uses three aligned regions (p0 exp/classes, p32 + p64
        # scratch) with small DMA copies realigning sub-slices; the
        # rad_classes accumulator lives in DRAM via accumulate-DMA.
        stage = apool.tile([128, 38, 140], F32, tag="stage")

        def mask_rows(t, rows, mcols):
            nc.vector.tensor_tensor(
                t[:, rows[0]:rows[1], :], t[:, rows[0]:rows[1], :],
                masks_sb[:, mcols[0]:mcols[1], None].to_broadcast(
                    (128, mcols[1] - mcols[0], W)),
                OP.mult,
            )

        def zero_cols(t, h, c0=146, row0=0):
            nc.gpsimd.memset(t[:, row0:row0 + h, c0:W], 0.0)

        def dbg_dump(ap):
            nc.sync.dma_start(out_d[:], ap)

        state = None
        state_rows = 0
        emb_prev = None
        for sp in range(4):
            xin = apool.tile([67, 65, W], MM_DT, tag="xin")
            nc.sync.dma_start(xin[:, 0:64, :], x_d[sp])
            nc.gpsimd.memset(xin[:, 64:65, :], 0.0)  # slack row: no NaNs
            # Tiles feeding SAME convs (emb, st, cA) carry a zeroed leading
            # row so the -1 column shift reads zero and PSUM writes stay
            # element-0 aligned.
            cur_flat, cur_rows = _flat(xin), 64
            for li in range(7):
                h_out = cur_rows - 2
                if li == 6:
                    out_t = epool.tile([128, 52, W], MM_DT, tag="emb")
                    lead = 1
                    nc.gpsimd.memset(out_t[:, 0:1, :], 0.0)
                    nc.gpsimd.memset(out_t[:, 51:52, :], 0.0)
                else:
                    tag = "cA" if li % 2 == 0 else "cB"
                    rows_alloc = 64 if tag == "cA" else 61
                    lead = 1 if tag == "cA" else 0
                    out_t = apool.tile([128, rows_alloc, W], MM_DT, tag=tag)
                    if sp == 0 and li < 2:
                        nc.gpsimd.memset(out_t[:, lead + h_out:rows_alloc, :], 0.0)
                wkey = f"e{li + 1}"
                _emit_conv(nc, psum, [cur_flat], ws[wkey], 1, 128, h_out,
                           out_t, lead, W, bias_ap(li, 128), "lrelu")
                if li < 6:
                    # zero wrap-around garbage columns so they never amplify
                    zero_cols(out_t, h_out, 158 - 2 * li, row0=lead)
                cur_flat, cur_rows = _flat(out_t)[:, lead * W:], h_out
            emb = out_t  # [128, 1+50+1, W], data rows [1, 51)
            zero_cols(emb, 50, row0=1)
            mask_rows(emb, (1, 4), (0, 3))
            mask_rows(emb, (48, 51), (3, 6))
            if debug_stage == 2 and sp == 0:
                dbgt = apool.tile([3, 34, 136], F32, tag="dbg")
                nc.vector.tensor_copy(out=dbgt[:], in_=emb[0:3, 1:35, 0:136])
                dbg_dump(dbgt[:])
                return nc

            # classifier (from emb rows [3,47) = tile rows [4,48))
            c1 = apool.tile([128, 64, W], MM_DT, tag="cA")
            _emit_conv(nc, psum, [_flat(emb)[:, 4 * W:]], ws["c1"], 1, 128, 42,
                       c1, 1, W, bias_ap(7, 128), "lrelu")
            zero_cols(c1, 42, 144, row0=1)
            c2 = apool.tile([128, 61, W], MM_DT, tag="cB")
            _emit_conv(nc, psum, [_flat(c1)[:, W:]], ws["c2"], 1, 128, 40,
                       c2, 0, W, bias_ap(8, 128), "lrelu")
            zero_cols(c2, 40, 142)
            _emit_conv(nc, psum, [_flat(c2)], ws["c3"], 1, 12, 38,
                       stage[0:12], 0, 140, bias_ap(9, 12), "linear")

            # softmax over 4 classes (channels replicated x3 by the conv
            # weights: partition 3c+ch holds logit c) and w_rad accumulate.
            # TensorTensor needs both SBUF inputs at the SAME base
            # partition, so folds ping-pong across three tiles at base 0.
            nc.scalar.activation(stage[0:12], stage[0:12], AF.Exp)
            smT = apool.tile([128, 38, 140], F32, tag="cB", name=f"smT{sp}")
            smU = apool.tile([128, 38, 140], F32, tag="cA", name=f"smU{sp}")
            nc.sync.dma_start(smT[0:6], stage[6:12])
            nc.vector.tensor_add(smU[0:6], stage[0:6], smT[0:6])
            nc.sync.dma_start(smT[0:3], smU[3:6])
            nc.vector.tensor_add(smT[32:35], smU[0:3], smT[0:3])
            nc.vector.reciprocal(smT[0:3], smT[32:35])
            nc.sync.dma_start(smT[3:6], smT[0:3])
            nc.sync.dma_start(smT[6:12], smT[0:6])
            nc.vector.tensor_tensor(stage[0:12], stage[0:12], smT[0:12], OP.mult)
            nc.sync.dma_start(smU[0:12], rad12_d[sp])
            nc.vector.tensor_tensor(stage[0:12], stage[0:12], smU[0:12], OP.mult)
            if debug_stage == 3 and sp == 0:
                dbgt = apool.tile([3, 34, 136], F32, tag="dbg")
                nc.vector.tensor_copy(out=dbgt[:], in_=stage[0:3, 0:34, 0:136])
                dbg_dump(dbgt[:])
                return nc
            # rad_classes accumulator lives at stage[64:76] (base-64 adds
            # against a base-64 staged addend in smT)
            if sp == 0:
                nc.sync.dma_start(stage[64:76], stage[0:12])
            else:
                nc.sync.dma_start(smT[64:76], stage[0:12])
                nc.vector.tensor_add(stage[64:76], stage[64:76], smT[64:76])
            if debug_stage == 35 and sp == 0:
                nc.sync.dma_start(out_d[:], stage[64:67, 0:34, 0:136])
                return nc

            # recurrent state update
            if sp == 0 or debug_stage == 45:
                state, state_rows, emb_prev = emb, 50, emb
            elif debug_stage == 46 and sp >= 2:
                pass  # keep state from sp1's single recurrent application
            else:
                n = state_rows
                eoff = 2 * (sp - 1)
                z = apool.tile([128, 64, W], MM_DT, tag="cA")
                nc.gpsimd.memset(z[:, 0:1, :], 0.0)
                _emit_conv(nc, psum,
                           [_same_in(state), _same_in(emb, 1 + eoff)],
                           ws["r1"], 2, 128, n - 2, z, 1, W,
                           bias_ap(10, 128), "lrelu")
                zero_cols(z, n - 2, row0=1)
                if sp == 1:
                    mask_rows(z, (1, 3), (1, 3))
                    mask_rows(z, (n - 3, n - 1), (3, 5))
                st_tag = "stA" if sp % 2 == 1 else "stB"
                st_new = apool.tile([128, 48, W], MM_DT, tag=st_tag)
                nc.gpsimd.memset(st_new[:, 0:1, :], 0.0)
                nc.gpsimd.memset(st_new[:, 1 + n - 4:48, :], 0.0)
                _emit_conv(nc, psum, [_same_in(z)], ws["r2"], 1, 128, n - 4,
                           st_new, 1, W, bias_ap(11, 128), "linear")
                zero_cols(st_new, n - 4, row0=1)
                if sp == 1:
                    mask_rows(st_new, (1, 2), (2, 3))
                    mask_rows(st_new, (n - 4, n - 3), (3, 4))
                state, state_rows = st_new, n - 4

        if debug_stage in (4, 45, 46):
            nc.sync.dma_start(out_d[:], stage[64:67, 0:34, 0:136])
            return nc
        if debug_stage == 5:
            dbgt = apool.tile([3, 34, 136], F32, tag="dbg")
            nc.vector.tensor_copy(out=dbgt[:], in_=state[0:3, 1:35, 0:136])
            dbg_dump(dbgt[:])
            return nc

        # kernel predictor: state [128, 38, W] -> kern [100, 34, 142]
        z = apool.tile([128, 64, W], MM_DT, tag="cA")
        _emit_conv(nc, psum, [_flat(state)[:, W:]], ws["k1"], 1, 128, 36,
                   z, 1, W, bias_ap(12, 128), "lrelu")
        zero_cols(z, 36, 144, row0=1)
        kern = apool.tile([100, 34, 142], F32, tag="cB")
        _emit_conv(nc, psum, [_flat(z)[:, W:]], ws["k2"], 1, 100, 34,
                   kern, 0, 142, bias_ap(13, 100), "linear")

        # softmax over the 25 taps of each class (channel j = 4*tap + c).
        # Partition-tree sums: both tensor_tensor inputs must share a base
        # partition, so each fold DMA-realigns operands to base 0 of two
        # scratch tiles (t2, t3) and ping-pongs.
        nc.scalar.activation(kern[0:100], kern[0:100], AF.Exp)
        t2 = apool.tile([128, 34, 142], F32, tag="cA")
        t3 = apool.tile([128, 34, 142], F32, tag="xin")
        nc.sync.dma_start(t2[0:32], kern[32:64])
        nc.vector.tensor_add(t3[0:32], kern[0:32], t2[0:32])      # t,t+8
        nc.sync.dma_start(t2[0:32], kern[64:96])
        nc.vector.tensor_add(t2[32:64], t3[0:32], t2[0:32])       # +t+16
        nc.sync.dma_start(t3[0:16], t2[32:48])
        nc.sync.dma_start(t2[0:16], t2[48:64])
        nc.vector.tensor_add(t3[32:48], t3[0:16], t2[0:16])       # u,u+4
        nc.sync.dma_start(t2[0:8], t3[32:40])
        nc.sync.dma_start(t3[0:8], t3[40:48])
        nc.vector.tensor_add(t2[32:40], t2[0:8], t3[0:8])         # u,u+2
        nc.sync.dma_start(t3[0:4], t2[32:36])
        nc.sync.dma_start(t2[0:4], t2[36:40])
        nc.vector.tensor_add(t3[32:36], t3[0:4], t2[0:4])         # u,u+1
        nc.sync.dma_start(t2[0:4], kern[96:100])
        nc.sync.dma_start(t3[0:4], t3[32:36])
        nc.vector.tensor_add(t2[32:36], t3[0:4], t2[0:4])         # + t=24
        nc.vector.reciprocal(t2[0:4], t2[32:36])
        nc.sync.dma_start(t2[4:8], t2[0:4])
        nc.sync.dma_start(t2[8:16], t2[0:8])
        nc.sync.dma_start(t2[16:32], t2[0:16])
        nc.sync.dma_start(t2[32:64], t2[0:32])
        nc.sync.dma_start(t2[64:96], t2[0:32])
        nc.sync.dma_start(t2[96:100], t2[0:4])
        nc.vector.tensor_tensor(kern[0:100], kern[0:100], t2[0:100], OP.mult)
        if debug_stage == 6:
            dbg_dump(kern[0:3, 0:34, 3:139])
            return nc

        # apply the per-pixel kernels: 25 taps in 3 rounds of <=10, with
        # rad_classes (from DRAM) and kerN replicated across 120 partitions
        acc = apool.tile([120, 34, 136], F32, tag="stA")
        for r in range(3):
            nt = 10 if r < 2 else 5
            nparts = 12 * nt
            radrep = apool.tile([120, 34, 136], F32, tag="xin")
            for u in range(nt):
                t = 10 * r + u
                dy, dx = divmod(t, 5)
                nc.sync.dma_start(
                    radrep[12 * u:12 * u + 12],
                    stage[64:76, dy:dy + 34, dx:dx + 136])
            krep = epool.tile([120, 34, 136], F32, tag="emb")
            for ch in range(3):
                nc.sync.dma_start(
                    krep[ch:ch + 3 * (4 * nt - 1) + 1:3],
                    kern[40 * r:40 * r + 4 * nt, :, 3:139])
            if r == 0:
                nc.vector.tensor_tensor(acc[0:nparts], krep[0:nparts],
                                        radrep[0:nparts], OP.mult)
            else:
                tmp = apool.tile([120, 34, 136], F32, tag="cA")
                nc.vector.tensor_tensor(tmp[0:nparts], krep[0:nparts],
                                        radrep[0:nparts], OP.mult)
                nc.vector.tensor_add(acc[0:nparts], acc[0:nparts], tmp[0:nparts])
        # reduce 120 partitions -> 3 channels with a selector matmul:
        # sel3[p, j] = (p % 3 == j), out[j, px] = sum_p sel3[p,j]*acc[p,px]
        outt = apool.tile([3, 34, 136], F32, tag="cB")
        acc_f = _flat(acc)
        out_f = _flat(outt)
        npx = 34 * 136
        c0 = 0
        while c0 < npx:
            n = min(480, npx - c0)
            pso = psum.tile([3, 3, W], F32, tag="ps", name=f"psr{c0}")
            pso_f = pso.rearrange("c h w -> c (h w)")
            nc.tensor.matmul(pso_f[:, :n], sel_sb[:], acc_f[:, c0:c0 + n],
                             start=True, stop=True)
            nc.vector.tensor_copy(out=out_f[:, c0:c0 + n], in_=pso_f[:, :n])
            c0 += n
        nc.sync.dma_start(out_d[:], outt[:])
    return nc


def _lhsT(w):
    """[Cout, Cin, 3, 3] fp32 -> [Cin, 9, Cout] in the matmul dtype."""
    w = np.asarray(w, np.float32)
    cout, cin = w.shape[0], w.shape[1]
    return np.ascontiguousarray(
        w.transpose(1, 2, 3, 0).reshape(cin, 9, cout)).astype(MM_NP)


def make_inputs(features, radiance, global_features, params):
    feats = np.asarray(features, np.float32)
    rad = np.asarray(radiance, np.float32)
    gf = np.asarray(global_features, np.float32)
    P = {k: [(np.asarray(w, np.float32), np.asarray(b, np.float32))
             for (w, b) in params[k]] for k in params}

    shared = {}
    for i, (w, b) in enumerate(P["embedding"]):
        shared[f"w_e{i + 1}"] = _lhsT(w)
    shared["w_c1"] = _lhsT(P["classifier"][0][0])
    shared["w_c2"] = _lhsT(P["classifier"][1][0])
    idx12 = np.array([0, 0, 0, 1, 1, 1, 2, 2, 2, 3, 3, 3])
    shared["w_c3"] = _lhsT(P["classifier"][2][0][idx12])
    wr = P["recurrent"][0][0]
    shared["w_r1"] = np.concatenate(
        [_lhsT(wr[:, 0:128]), _lhsT(wr[:, 128:256])], axis=1)
    shared["w_r2"] = _lhsT(P["recurrent"][1][0])
    shared["w_k1"] = _lhsT(P["kernel_predictor"][0][0])
    perm = np.array([c * 25 + t for t in range(25) for c in range(4)])
    shared["w_k2"] = _lhsT(P["kernel_predictor"][1][0][perm])
    sel = np.zeros((120, 3), np.float32)
    for p in range(120):
        sel[p, p % 3] = 1.0
    shared["sel3"] = sel

    biases = np.zeros((128, 14), np.float32)
    for i in range(7):
        biases[:, i] = P["embedding"][i][1]
    biases[:, 7] = P["classifier"][0][1]
    biases[:, 8] = P["classifier"][1][1]
    biases[0:12, 9] = P["classifier"][2][1][idx12]
    biases[:, 10] = P["recurrent"][0][1]
    biases[:, 11] = P["recurrent"][1][1]
    biases[:, 12] = P["kernel_predictor"][0][1]
    biases[0:100, 13] = P["kernel_predictor"][1][1][perm]
    shared["biases"] = biases

    in_maps = []
    ch_idx = np.array([0, 1, 2, 0, 1, 2, 0, 1, 2, 0, 1, 2])
    for b in range(2):
        fp = np.zeros((4, 64, 166, W), np.float32)
        fp[:, :, 3:163] = feats[b]
        rp = np.zeros((4, 3, 166, W), np.float32)
        rp[:, :, 3:163] = rad[b]
        for k in range(NB):
            o0 = OB * k
            x = np.empty((4, 67, 64, W), MM_NP)
            x[:, 0:3] = gf[b, :, 0, 0][None, :, None, None].astype(MM_NP)
            x[:, 3:67] = fp[:, :, o0:o0 + 64, :].transpose(0, 1, 2, 3).astype(MM_NP)
            block = rp[:, :, o0 + 13:o0 + 51, 10:150]  # [4, 3, 38, 140]
            rad12 = np.ascontiguousarray(block[:, ch_idx] * 0.25, np.float32)
            g = np.arange(6)
            gl = np.concatenate([o0 - 3 + g[:3], o0 + 44 + g[:3]])
            mvals = ((gl >= 0) & (gl < 146)).astype(MM_NP)
            masks = np.broadcast_to(mvals, (128, 6)).copy()
            m = dict(shared)
            m["x"] = x
            m["rad12"] = rad12
            m["masks"] = masks
            in_maps.append(m)
    return in_maps


_NC_CACHE = {}


def get_program():
    if "nc" not in _NC_CACHE:
        _NC_CACHE["nc"] = build_program()
    return _NC_CACHE["nc"]


class Runner:
    """Compiles the SPMD program once; repeated .run() calls reuse the
    jitted sharded executable (no neuronxcc recompile per call)."""

    def __init__(self, nc, n_cores=8):
        import jax
        from jax.sharding import Mesh, PartitionSpec
        from jax.experimental.shard_map import shard_map
        import concourse.mybir as mb
        from concourse import bass2jax

        bass2jax.install_neuronx_cc_hook()
        self.nc = nc
        self.n_cores = n_cores
        part_name = (nc.partition_id_tensor.name
                     if nc.partition_id_tensor else None)
        in_names, out_names, out_avals, zero_outs = [], [], [], []
        for alloc in nc.m.functions[0].allocations:
            if not isinstance(alloc, mb.MemoryLocationSet):
                continue
            name = alloc.memorylocations[0].name
            if alloc.kind == "ExternalInput":
                if name != part_name:
                    in_names.append(name)
            elif alloc.kind == "ExternalOutput":
                out_names.append(name)
                shape = tuple(alloc.tensor_shape)
                dtype = mb.dt.np(alloc.dtype)
                out_avals.append(jax.core.ShapedArray(shape, dtype))
                zero_outs.append(np.zeros(shape, dtype))
        self.in_names = list(in_names)
        self.out_names = out_names
        self.out_avals = out_avals
        self.zero_outs = zero_outs
        n_params = len(in_names)
        n_outs = len(out_avals)
        all_names = in_names + out_names
        if part_name is not None:
            all_names = all_names + [part_name]

        def _body(*args):
            operands = list(args)
            if part_name is not None:
                operands.append(bass2jax.partition_id_tensor())
            outs = bass2jax._bass_exec_p.bind(
                *operands,
                out_avals=tuple(out_avals),
                in_names=tuple(all_names),
                out_names=tuple(out_names),
                lowering_input_output_aliases=(),
                sim_require_finite=True,
                sim_require_nnan=True,
                nc=nc,
            )
            return tuple(outs)

        devices = jax.devices()[:n_cores]
        mesh = Mesh(np.asarray(devices), ("core",))
        in_specs = (PartitionSpec("core"),) * (n_params + n_outs)
        out_specs = (PartitionSpec("core"),) * n_outs
        self._fn = jax.jit(
            shard_map(_body, mesh=mesh, in_specs=in_specs,
                      out_specs=out_specs, check_rep=False),
            donate_argnums=tuple(range(n_params, n_params + n_outs)),
            keep_unused=True,
        )

    def run_raw(self, concat_in):
        concat_zeros = [
            np.zeros((self.n_cores * z.shape[0], *z.shape[1:]), z.dtype)
            for z in self.zero_outs
        ]
        return self._fn(*concat_in, *concat_zeros)

    def concat_inputs(self, in_maps):
        return [
            np.concatenate([np.asarray(m[nm]) for m in in_maps], axis=0)
            for nm in self.in_names
        ]

    def run(self, in_maps):
        out_arrs = self.run_raw(self.concat_inputs(in_maps))
        n = self.n_cores
        return [
            {nm: np.asarray(out_arrs[i]).reshape(n, *self.out_avals[i].shape)[c]
             for i, nm in enumerate(self.out_names)}
            for c in range(n)
        ]


def get_runner():
    if "runner" not in _NC_CACHE:
        _NC_CACHE["runner"] = Runner(get_program())
    return _NC_CACHE["runner"]


def run(in_maps, **kw):
    from concourse.bass_utils import run_bass_kernel_spmd
    nc = get_program()
    return run_bass_kernel_spmd(nc, in_maps, core_ids=list(range(8)), **kw)


def kernel(features, radiance, global_features, params):
    in_maps = make_inputs(features, radiance, global_features, params)
    results = get_runner().run(in_maps)
    out = np.zeros((2, 3, 136, 136), np.float32)
    for b in range(2):
        for k in range(NB):
            out[b, :, OB * k:OB * (k + 1), :] = results[b * NB + k]["out"]
    return out


if __name__ == "__main__":
    import json
    nc = build_program()
    print("built OK, instructions:",
          sum(len(blk.instructions) for blk in []) or "n/a")


# revision 42
# speedup vs baseline: 675.1403x; 675.1403x over previous
# Trainium2 Bass kernel for nn_Binning (KPN denoiser-style net).
#
# Strategy: 8-way shard = batch(2) x 4 horizontal bands of the output.
# Each core computes a 34-row band of the final 136x136 output from a
# 64-row slice of the (row-padded) input, fully replicating the small conv
# weights. All convs run as 9 shifted matmuls accumulating in PSUM
# (channels on partitions, flattened rows*cols on the free axis, full
# 160-wide rows with wrap-around garbage columns that are provably never
# read for valid outputs). The recurrent SAME convs rely on zeroed
# cols >=146 and per-core row masks so band-local VALID convs reproduce
# global zero padding semantics.
import os
import sys

import numpy as np

for _p in ("/opt/trn_rl_repo",):
    if os.path.isdir(_p) and _p not in sys.path:
        sys.path.insert(0, _p)

import ml_dtypes

import concourse.bass as bass
import concourse.mybir as mybir
import concourse.tile as tile
from concourse.vector_clock import ScopedClock

W = 160
OB = 34          # output rows per band
NB = 4           # bands per batch element
LEAKY = 0.01
F32 = mybir.dt.float32
MM_DT = mybir.dt.float16
MM_NP = np.float16
AF = mybir.ActivationFunctionType
OP = mybir.AluOpType
_DISABLE_SAME = False  # debug: drop the -1 column shift (wrong numerics)


class _TC(tile.TileContext):
    """TileContext that spreads multi-semaphore waits across single-wait
    same-engine NOPs (this walrus build allows only one sync-wait per
    instruction)."""

    def _split_multi_waits(self):
        nc = self.nc
        cnt = [0]
        for fn in nc.m.functions:
            for blk in fn.blocks:
                insts = blk.instructions
                need = any(
                    i.sync_info is not None and i.sync_info.on_wait
                    and len(i.sync_info.on_wait) > 1 for i in insts)
                if not need:
                    continue
                new = []
                for i in insts:
                    si = i.sync_info
                    if si is not None and si.on_wait and len(si.on_wait) > 1:
                        waits = list(si.on_wait)
                        for w in waits[:-1]:
                            cnt[0] += 1
                            nop = mybir.InstNoOp(
                                name=f"I-waitsplit-{cnt[0]}", ins=[], outs=[])
                            nop.engine = i.engine
                            nop.sync_info = mybir.SyncInfo(
                                on_wait=[w], on_update=[])
                            nc.register_instruction(nop, overwrite=True)
                            new.append(nop)
                        si.on_wait = [waits[-1]]
                    new.append(i)
                while len(blk.instructions):
                    blk.instructions.pop()
                for i in new:
                    blk.instructions.append(i)

    def _drain_and_barrier(self, tick_clock, wait_clock):
        nc = self.nc
        self._split_multi_waits()
        drain_inst = nc.sync.drain()
        wait_clock.add_sem_waits(
            drain_inst.ins, ScopedClock({None: tick_clock.global_clock})
        )
        si = drain_inst.ins.sync_info
        waits = list(si.on_wait or [])
        if len(waits) > 1:
            si.on_wait = [waits[-1]]
            bb = nc.cur_bb.bb
            insts = bb.instructions
            assert insts[-1] is drain_inst.ins
            insts.pop()
            for w in waits[:-1]:
                n = nc.sync.nop(nofuse=True, hint="drain_wait_split")
                n.ins.sync_info = mybir.SyncInfo(on_wait=[w], on_update=[])
            insts.append(drain_inst.ins)
        nc.all_engine_barrier()
        popped = nc._tile_sem_poison_stack.pop()
        assert popped is self._sem_poison
        nc.clear_and_free_semaphores(list(self.sems.allocated().values()))
        nc.all_engine_barrier()


def _emit_conv(nc, psum_pool, ins, w_sb, n_kg, cout, h_out, out_base, out_row0,
               ncols, bias_ap, act):
    """One 3x3 conv layer: channels on partitions, 9 shifted matmuls
    accumulated in PSUM, 3 output rows (480 elems) per PSUM bank, 8 banks
    per round. `ins` = per-k-group flat [Cin, >= (h_in+1)*W] SBUF APs.
    For SAME-column convs the caller passes `ins` with its base already
    shifted by -1 element into the zeroed lead row. Evacuation: ACT
    Lrelu(psum+bias) or DVE psum+bias -> out_base[:, out_row0+r:, 0:ncols]."""
    r0 = 0
    while r0 < h_out:
        rem = h_out - r0
        nb = min(8, (rem + 2) // 3)
        banks = []
        for b in range(nb):
            rows_b = min(3, rem - 3 * b)
            if rows_b <= 0:
                break
            banks.append((b, rows_b))
        psums = [psum_pool.tile([cout, 3, W], F32, tag="ps", name=f"ps{r0}_{i}")
                 for i in range(len(banks))]
        for kg in range(n_kg):
            for t9 in range(9):
                dy, dx = divmod(t9, 3)
                lhsT = w_sb[:, kg * 9 + t9, :]
                start = kg == 0 and t9 == 0
                stop = kg == n_kg - 1 and t9 == 8
                for (b, rows_b), ps in zip(banks, psums):
                    off = (r0 + 3 * b + dy) * W + dx
                    n = rows_b * W
                    ps_flat = ps.rearrange("c h w -> c (h w)")
                    nc.tensor.matmul(
                        ps_flat[:, :n], lhsT, ins[kg][:, off:off + n],
                        start=start, stop=stop,
                    )
        for (b, rows_b), ps in zip(banks, psums):
            r = out_row0 + r0 + 3 * b
            o = out_base[:, r:r + rows_b, 0:ncols]
            if act == "lrelu":
                nc.scalar.activation(
                    o, ps[:, :rows_b, 0:ncols], AF.Lrelu,
                    bias=bias_ap, scale=1.0, alpha=LEAKY,
                )
            else:
                nc.vector.tensor_scalar(
                    o, ps[:, :rows_b, 0:ncols], bias_ap, None, OP.add
                )
        r0 += 3 * len(banks)


def _flat(ap):
    return ap.rearrange("c h w -> c (h w)")


def _same_in(ap, lead_rows=1):
    """Flat view starting one element before data row `lead_rows` (reads
    the zeroed lead row's last column for the SAME conv's column -1)."""
    sh = 0 if _DISABLE_SAME else 1
    return _flat(ap)[:, lead_rows * W - sh:]


def build_program(debug_stage=None):
    nc = bass.Bass("TRN2")

    # --- DRAM I/O ---
    x_d = nc.dram_tensor("x", (4, 67, 64, W), MM_DT, kind="ExternalInput")
    rad12_d = nc.dram_tensor("rad12", (4, 12, 38, 140), F32, kind="ExternalInput")
    masks_d = nc.dram_tensor("masks", (128, 6), MM_DT, kind="ExternalInput")
    bias_d = nc.dram_tensor("biases", (128, 14), F32, kind="ExternalInput")
    wd = {}
    wd["e1"] = nc.dram_tensor("w_e1", (67, 9, 128), MM_DT, kind="ExternalInput")
    for i in range(2, 8):
        wd[f"e{i}"] = nc.dram_tensor(f"w_e{i}", (128, 9, 128), MM_DT, kind="ExternalInput")
    wd["c1"] = nc.dram_tensor("w_c1", (128, 9, 128), MM_DT, kind="ExternalInput")
    wd["c2"] = nc.dram_tensor("w_c2", (128, 9, 128), MM_DT, kind="ExternalInput")
    wd["c3"] = nc.dram_tensor("w_c3", (128, 9, 12), MM_DT, kind="ExternalInput")
    wd["r1"] = nc.dram_tensor("w_r1", (128, 18, 128), MM_DT, kind="ExternalInput")
    wd["r2"] = nc.dram_tensor("w_r2", (128, 9, 128), MM_DT, kind="ExternalInput")
    wd["k1"] = nc.dram_tensor("w_k1", (128, 9, 128), MM_DT, kind="ExternalInput")
    wd["k2"] = nc.dram_tensor("w_k2", (128, 9, 100), MM_DT, kind="ExternalInput")
    sel_d = nc.dram_tensor("sel3", (120, 3), F32, kind="ExternalInput")
    out_d = nc.dram_tensor("out", (3, OB, 136), F32, kind="ExternalOutput")

    from contextlib import ExitStack
    with _TC(nc) as tc, ExitStack() as ctx:
        wpool = ctx.enter_context(tc.tile_pool(name="w", bufs=1))
        apool = ctx.enter_context(tc.tile_pool(name="a", bufs=1))
        epool = ctx.enter_context(tc.tile_pool(name="e", bufs=2))
        psum = ctx.enter_context(tc.tile_pool(name="p", bufs=8, space="PSUM"))

        # --- load constants ---
        ws = {}
        for key, d in wd.items():
            t = wpool.tile(list(d.shape), MM_DT, tag=f"w_{key}")
            nc.sync.dma_start(t[:], d[:])
            ws[key] = t
        bias_sb = wpool.tile([128, 14], F32, tag="bias")
        nc.sync.dma_start(bias_sb[:], bias_d[:])
        masks_sb = wpool.tile([128, 6], MM_DT, tag="masks")
        nc.sync.dma_start(masks_sb[:], masks_d[:])
        sel_sb = wpool.tile([120, 3], F32, tag="sel")
        nc.sync.dma_start(sel_sb[:], sel_d[:])

        def bias_ap(col, cout):
            return bias_sb[0:cout, col:col + 1]

        # stage tile: f32 scratch for the per-sp softmax/w_rad stage.
        # Compute-engine APs may only start at partitions 0/32/64, so the
        # layout uses three aligned regions (p0 exp/classes, p32 + p64
        # scratch) with small DMA copies realigning sub-slices; the
        # rad_classes accumulator lives in DRAM via accumulate-DMA.
        stage = apool.tile([128, 38, 140], F32, tag="stage")

        def mask_rows(t, rows, mcols):
            nc.vector.tensor_tensor(
                t[:, rows[0]:rows[1], :], t[:, rows[0]:rows[1], :],
                masks_sb[:, mcols[0]:mcols[1], None].to_broadcast(
                    (128, mcols[1] - mcols[0], W)),
                OP.mult,
            )

        def zero_cols(t, h, c0=146, row0=0):
            nc.gpsimd.memset(t[:, row0:row0 + h, c0:W], 0.0)

        def dbg_dump(ap):
            nc.sync.dma_start(out_d[:], ap)

        state = None
        state_rows = 0
        emb_prev = None
        for sp in range(4):
            xin = apool.tile([67, 65, W], MM_DT, tag="xin")
            nc.sync.dma_start(xin[:, 0:64, :], x_d[sp])
            nc.gpsimd.memset(xin[:, 64:65, :], 0.0)  # slack row: no NaNs
            # Tiles feeding SAME convs (emb, st, cA) carry a zeroed leading
            # row so the -1 column shift reads zero and PSUM writes stay
            # element-0 aligned.
            cur_flat, cur_rows = _flat(xin), 64
            for li in range(7):
                h_out = cur_rows - 2
                if li == 6:
                    out_t = epool.tile([128, 52, W], MM_DT, tag="emb")
                    lead = 1
                    nc.gpsimd.memset(out_t[:, 0:1, :], 0.0)
                    nc.gpsimd.memset(out_t[:, 51:52, :], 0.0)
                else:
                    tag = "cA" if li % 2 == 0 else "cB"
                    rows_alloc = 64 if tag == "cA" else 61
                    lead = 1 if tag == "cA" else 0
                    out_t = apool.tile([128, rows_alloc, W], MM_DT, tag=tag)
                    if sp == 0 and li < 2:
                        nc.gpsimd.memset(out_t[:, lead + h_out:rows_alloc, :], 0.0)
                wkey = f"e{li + 1}"
                _emit_conv(nc, psum, [cur_flat], ws[wkey], 1, 128, h_out,
                           out_t, lead, W, bias_ap(li, 128), "lrelu")
                if li < 6:
                    # zero wrap-around garbage columns so they never amplify
                    zero_cols(out_t, h_out, 158 - 2 * li, row0=lead)
                cur_flat, cur_rows = _flat(out_t)[:, lead * W:], h_out
            emb = out_t  # [128, 1+50+1, W], data rows [1, 51)
            zero_cols(emb, 50, row0=1)
            mask_rows(emb, (1, 4), (0, 3))
            mask_rows(emb, (48, 51), (3, 6))
            if debug_stage == 2 and sp == 0:
                dbgt = apool.tile([3, 34, 136], F32, tag="dbg")
                nc.vector.tensor_copy(out=dbgt[:], in_=emb[0:3, 1:35, 0:136])
                dbg_dump(dbgt[:])
                return nc

            # classifier (from emb rows [3,47) = tile rows [4,48))
            c1 = apool.tile([128, 64, W], MM_DT, tag="cA")
            _emit_conv(nc, psum, [_flat(emb)[:, 4 * W:]], ws["c1"], 1, 128, 42,
                       c1, 1, W, bias_ap(7, 128), "lrelu")
            zero_cols(c1, 42, 144, row0=1)
            c2 = apool.tile([128, 61, W], MM_DT, tag="cB")
            _emit_conv(nc, psum, [_flat(c1)[:, W:]], ws["c2"], 1, 128, 40,
                       c2, 0, W, bias_ap(8, 128), "lrelu")
            zero_cols(c2, 40, 142)
            _emit_conv(nc, psum, [_flat(c2)], ws["c3"], 1, 12, 38,
                       stage[0:12], 0, 140, bias_ap(9, 12), "linear")

            # softmax over 4 classes (channels replicated x3 by the conv
            # weights: partition 3c+ch holds logit c) and w_rad accumulate.
            # TensorTensor needs both SBUF inputs at the SAME base
            # partition, so folds ping-pong across three tiles at base 0.
            nc.scalar.activation(stage[0:12], stage[0:12], AF.Exp)
            smT = apool.tile([128, 38, 140], F32, tag="cB", name=f"smT{sp}")
            smU = apool.tile([128, 38, 140], F32, tag="cA", name=f"smU{sp}")
            nc.sync.dma_start(smT[0:6], stage[6:12])
            nc.vector.tensor_add(smU[0:6], stage[0:6], smT[0:6])
            nc.sync.dma_start(smT[0:3], smU[3:6])
            nc.vector.tensor_add(smT[32:35], smU[0:3], smT[0:3])
            nc.vector.reciprocal(smT[0:3], smT[32:35])
            nc.sync.dma_start(smT[3:6], smT[0:3])
            nc.sync.dma_start(smT[6:12], smT[0:6])
            nc.vector.tensor_tensor(stage[0:12], stage[0:12], smT[0:12], OP.mult)
            nc.sync.dma_start(smU[0:12], rad12_d[sp])
            nc.vector.tensor_tensor(stage[0:12], stage[0:12], smU[0:12], OP.mult)
            if debug_stage == 3 and sp == 0:
                dbgt = apool.tile([3, 34, 136], F32, tag="dbg")
                nc.vector.tensor_copy(out=dbgt[:], in_=stage[0:3, 0:34, 0:136])
                dbg_dump(dbgt[:])
                return nc
            # rad_classes accumulator lives at stage[64:76] (base-64 adds
            # against a base-64 staged addend in smT)
            if sp == 0:
                nc.sync.dma_start(stage[64:76], stage[0:12])
            else:
                nc.sync.dma_start(smT[64:76], stage[0:12])
                nc.vector.tensor_add(stage[64:76], stage[64:76], smT[64:76])
            if debug_stage == 35 and sp == 0:
                nc.sync.dma_start(out_d[:], stage[64:67, 0:34, 0:136])
                return nc

            # recurrent state update
            if sp == 0 or debug_stage == 45:
                state, state_rows, emb_prev = emb, 50, emb
            elif debug_stage == 46 and sp >= 2:
                pass  # keep state from sp1's single recurrent application
            else:
                n = state_rows
                eoff = 2 * (sp - 1)
                z = apool.tile([128, 64, W], MM_DT, tag="cA")
                nc.gpsimd.memset(z[:, 0:1, :], 0.0)
                _emit_conv(nc, psum,
                           [_same_in(state), _same_in(emb, 1 + eoff)],
                           ws["r1"], 2, 128, n - 2, z, 1, W,
                           bias_ap(10, 128), "lrelu")
                zero_cols(z, n - 2, row0=1)
                if sp == 1:
                    mask_rows(z, (1, 3), (1, 3))
                    mask_rows(z, (n - 3, n - 1), (3, 5))
                st_tag = "stA" if sp % 2 == 1 else "stB"
                st_new = apool.tile([128, 48, W], MM_DT, tag=st_tag)
                nc.gpsimd.memset(st_new[:, 0:1, :], 0.0)
                nc.gpsimd.memset(st_new[:, 1 + n - 4:48, :], 0.0)
                _emit_conv(nc, psum, [_same_in(z)], ws["r2"], 1, 128, n - 4,
                           st_new, 1, W, bias_ap(11, 128), "linear")
                zero_cols(st_new, n - 4, row0=1)
                if sp == 1:
                    mask_rows(st_new, (1, 2), (2, 3))
                    mask_rows(st_new, (n - 4, n - 3), (3, 4))
                state, state_rows = st_new, n - 4

        if debug_stage in (4, 45, 46):
            nc.sync.dma_start(out_d[:], stage[64:67, 0:34, 0:136])
            return nc
        if debug_stage == 5:
            dbgt = apool.tile([3, 34, 136], F32, tag="dbg")
            nc.vector.tensor_copy(out=dbgt[:], in_=state[0:3, 1:35, 0:136])
            dbg_dump(dbgt[:])
            return nc

        # kernel predictor: state [128, 38, W] -> kern [100, 34, 142]
        z = apool.tile([128, 64, W], MM_DT, tag="cA")
        _emit_conv(nc, psum, [_flat(state)[:, W:]], ws["k1"], 1, 128, 36,
                   z, 1, W, bias_ap(12, 128), "lrelu")
        zero_cols(z, 36, 144, row0=1)
        kern = apool.tile([100, 34, 142], F32, tag="cB")
        _emit_conv(nc, psum, [_flat(z)[:, W:]], ws["k2"], 1, 100, 34,
                   kern, 0, 142, bias_ap(13, 100), "linear")

        # softmax over the 25 taps of each class (channel j = 4*tap + c).
        # Partition-tree sums: both tensor_tensor inputs must share a base
        # partition, so each fold DMA-realigns operands to base 0 of two
        # scratch tiles (t2, t3) and ping-pongs.
        nc.scalar.activation(kern[0:100], kern[0:100], AF.Exp)
        t2 = apool.tile([128, 34, 142], F32, tag="cA")
        t3 = apool.tile([128, 34, 142], F32, tag="xin")
        nc.sync.dma_start(t2[0:32], kern[32:64])
        nc.vector.tensor_add(t3[0:32], kern[0:32], t2[0:32])      # t,t+8
        nc.sync.dma_start(t2[0:32], kern[64:96])
        nc.vector.tensor_add(t2[32:64], t3[0:32], t2[0:32])       # +t+16
        nc.sync.dma_start(t3[0:16], t2[32:48])
        nc.sync.dma_start(t2[0:16], t2[48:64])
        nc.vector.tensor_add(t3[32:48], t3[0:16], t2[0:16])       # u,u+4
        nc.sync.dma_start(t2[0:8], t3[32:40])
        nc.sync.dma_start(t3[0:8], t3[40:48])
        nc.vector.tensor_add(t2[32:40], t2[0:8], t3[0:8])         # u,u+2
        nc.sync.dma_start(t3[0:4], t2[32:36])
        nc.sync.dma_start(t2[0:4], t2[36:40])
        nc.vector.tensor_add(t3[32:36], t3[0:4], t2[0:4])         # u,u+1
        nc.sync.dma_start(t2[0:4], kern[96:100])
        nc.sync.dma_start(t3[0:4], t3[32:36])
        nc.vector.tensor_add(t2[32:36], t3[0:4], t2[0:4])         # + t=24
        nc.vector.reciprocal(t2[0:4], t2[32:36])
        nc.sync.dma_start(t2[4:8], t2[0:4])
        nc.sync.dma_start(t2[8:16], t2[0:8])
        nc.sync.dma_start(t2[16:32], t2[0:16])
        nc.sync.dma_start(t2[32:64], t2[0:32])
        nc.sync.dma_start(t2[64:96], t2[0:32])
        nc.sync.dma_start(t2[96:100], t2[0:4])
        nc.vector.tensor_tensor(kern[0:100], kern[0:100], t2[0:100], OP.mult)
        if debug_stage == 6:
            dbg_dump(kern[0:3, 0:34, 3:139])
            return nc

        # apply the per-pixel kernels: 25 taps in 3 rounds of <=10, with
        # rad_classes (from DRAM) and kerN replicated across 120 partitions
        acc = apool.tile([120, 34, 136], F32, tag="stA")
        for r in range(3):
            nt = 10 if r < 2 else 5
            nparts = 12 * nt
            radrep = apool.tile([120, 34, 136], F32, tag="xin")
            for u in range(nt):
                t = 10 * r + u
                dy, dx = divmod(t, 5)
                nc.sync.dma_start(
                    radrep[12 * u:12 * u + 12],
                    stage[64:76, dy:dy + 34, dx:dx + 136])
            krep = epool.tile([120, 34, 136], F32, tag="emb")
            for ch in range(3):
                nc.sync.dma_start(
                    krep[ch:ch + 3 * (4 * nt - 1) + 1:3],
                    kern[40 * r:40 * r + 4 * nt, :, 3:139])
            if r == 0:
                nc.vector.tensor_tensor(acc[0:nparts], krep[0:nparts],
                                        radrep[0:nparts], OP.mult)
            else:
                tmp = apool.tile([120, 34, 136], F32, tag="cA")
                nc.vector.tensor_tensor(tmp[0:nparts], krep[0:nparts],
                                        radrep[0:nparts], OP.mult)
                nc.vector.tensor_add(acc[0:nparts], acc[0:nparts], tmp[0:nparts])
        # reduce 120 partitions -> 3 channels with a selector matmul:
        # sel3[p, j] = (p % 3 == j), out[j, px] = sum_p sel3[p,j]*acc[p,px]
        outt = apool.tile([3, 34, 136], F32, tag="cB")
        acc_f = _flat(acc)
        out_f = _flat(outt)
        npx = 34 * 136
        c0 = 0
        while c0 < npx:
            n = min(480, npx - c0)
            pso = psum.tile([3, 3, W], F32, tag="ps", name=f"psr{c0}")
            pso_f = pso.rearrange("c h w -> c (h w)")
            nc.tensor.matmul(pso_f[:, :n], sel_sb[:], acc_f[:, c0:c0 + n],
                             start=True, stop=True)
            nc.vector.tensor_copy(out=out_f[:, c0:c0 + n], in_=pso_f[:, :n])
            c0 += n
        nc.sync.dma_start(out_d[:], outt[:])
    return nc


def _lhsT(w):
    """[Cout, Cin, 3, 3] fp32 -> [Cin, 9, Cout] in the matmul dtype."""
    w = np.asarray(w, np.float32)
    cout, cin = w.shape[0], w.shape[1]
    return np.ascontiguousarray(
        w.transpose(1, 2, 3, 0).reshape(cin, 9, cout)).astype(MM_NP)


def make_inputs(features, radiance, global_features, params):
    feats = np.asarray(features, np.float32)
    rad = np.asarray(radiance, np.float32)
    gf = np.asarray(global_features, np.float32)
    P = {k: [(np.asarray(w, np.float32), np.asarray(b, np.float32))
             for (w, b) in params[k]] for k in params}

    shared = {}
    for i, (w, b) in enumerate(P["embedding"]):
        shared[f"w_e{i + 1}"] = _lhsT(w)
    shared["w_c1"] = _lhsT(P["classifier"][0][0])
    shared["w_c2"] = _lhsT(P["classifier"][1][0])
    idx12 = np.array([0, 0, 0, 1, 1, 1, 2, 2, 2, 3, 3, 3])
    shared["w_c3"] = _lhsT(P["classifier"][2][0][idx12])
    wr = P["recurrent"][0][0]
    shared["w_r1"] = np.concatenate(
        [_lhsT(wr[:, 0:128]), _lhsT(wr[:, 128:256])], axis=1)
    shared["w_r2"] = _lhsT(P["recurrent"][1][0])
    shared["w_k1"] = _lhsT(P["kernel_predictor"][0][0])
    perm = np.array([c * 25 + t for t in range(25) for c in range(4)])
    shared["w_k2"] = _lhsT(P["kernel_predictor"][1][0][perm])
    sel = np.zeros((120, 3), np.float32)
    for p in range(120):
        sel[p, p % 3] = 1.0
    shared["sel3"] = sel

    biases = np.zeros((128, 14), np.float32)
    for i in range(7):
        biases[:, i] = P["embedding"][i][1]
    biases[:, 7] = P["classifier"][0][1]
    biases[:, 8] = P["classifier"][1][1]
    biases[0:12, 9] = P["classifier"][2][1][idx12]
    biases[:, 10] = P["recurrent"][0][1]
    biases[:, 11] = P["recurrent"][1][1]
    biases[:, 12] = P["kernel_predictor"][0][1]
    biases[0:100, 13] = P["kernel_predictor"][1][1][perm]
    shared["biases"] = biases

    in_maps = []
    ch_idx = np.array([0, 1, 2, 0, 1, 2, 0, 1, 2, 0, 1, 2])
    for b in range(2):
        fp = np.zeros((4, 64, 166, W), np.float32)
        fp[:, :, 3:163] = feats[b]
        rp = np.zeros((4, 3, 166, W), np.float32)
        rp[:, :, 3:163] = rad[b]
        for k in range(NB):
            o0 = OB * k
            x = np.empty((4, 67, 64, W), MM_NP)
            x[:, 0:3] = gf[b, :, 0, 0][None, :, None, None].astype(MM_NP)
            x[:, 3:67] = fp[:, :, o0:o0 + 64, :].transpose(0, 1, 2, 3).astype(MM_NP)
            block = rp[:, :, o0 + 13:o0 + 51, 10:150]  # [4, 3, 38, 140]
            rad12 = np.ascontiguousarray(block[:, ch_idx] * 0.25, np.float32)
            g = np.arange(6)
            gl = np.concatenate([o0 - 3 + g[:3], o0 + 44 + g[:3]])
            mvals = ((gl >= 0) & (gl < 146)).astype(MM_NP)
            masks = np.broadcast_to(mvals, (128, 6)).copy()
            m = dict(shared)
            m["x"] = x
            m["rad12"] = rad12
            m["masks"] = masks
            in_maps.append(m)
    return in_maps


_NC_CACHE = {}


def get_program():
    if "nc" not in _NC_CACHE:
        _NC_CACHE["nc"] = build_program()
    return _NC_CACHE["nc"]


class Runner:
    """Compiles the SPMD program once; repeated .run() calls reuse the
    jitted sharded executable (no neuronxcc recompile per call)."""

    def __init__(self, nc, n_cores=8):
        import jax
        from jax.sharding import Mesh, PartitionSpec
        from jax.experimental.shard_map import shard_map
        import concourse.mybir as mb
        from concourse import bass2jax

        bass2jax.install_neuronx_cc_hook()
        self.nc = nc
        self.n_cores = n_cores
        part_name = (nc.partition_id_tensor.name
                     if nc.partition_id_tensor else None)
        in_names, out_names, out_avals, zero_outs = [], [], [], []
        for alloc in nc.m.functions[0].allocations:
            if not isinstance(alloc, mb.MemoryLocationSet):
                continue
            name = alloc.memorylocations[0].name
            if alloc.kind == "ExternalInput":
                if name != part_name:
                    in_names.append(name)
            elif alloc.kind == "ExternalOutput":
                out_names.append(name)
                shape = tuple(alloc.tensor_shape)
                dtype = mb.dt.np(alloc.dtype)
                out_avals.append(jax.core.ShapedArray(shape, dtype))
                zero_outs.append(np.zeros(shape, dtype))
        self.in_names = list(in_names)
        self.out_names = out_names
        self.out_avals = out_avals
        self.zero_outs = zero_outs
        n_params = len(in_names)
        n_outs = len(out_avals)
        all_names = in_names + out_names
        if part_name is not None:
            all_names = all_names + [part_name]

        def _body(*args):
            operands = list(args)
            if part_name is not None:
                operands.append(bass2jax.partition_id_tensor())
            outs = bass2jax._bass_exec_p.bind(
                *operands,
                out_avals=tuple(out_avals),
                in_names=tuple(all_names),
                out_names=tuple(out_names),
                lowering_input_output_aliases=(),
                sim_require_finite=True,
                sim_require_nnan=True,
                nc=nc,
            )
            return tuple(outs)

        devices = jax.devices()[:n_cores]
        mesh = Mesh(np.asarray(devices), ("core",))
        in_specs = (PartitionSpec("core"),) * (n_params + n_outs)
        out_specs = (PartitionSpec("core"),) * n_outs
        self._fn = jax.jit(
            shard_map(_body, mesh=mesh, in_specs=in_specs,
                      out_specs=out_specs, check_rep=False),
            donate_argnums=tuple(range(n_params, n_params + n_outs)),
            keep_unused=True,
        )

    def run_raw(self, concat_in):
        concat_zeros = [
            np.zeros((self.n_cores * z.shape[0], *z.shape[1:]), z.dtype)
            for z in self.zero_outs
        ]
        return self._fn(*concat_in, *concat_zeros)

    def concat_inputs(self, in_maps):
        return [
            np.concatenate([np.asarray(m[nm]) for m in in_maps], axis=0)
            for nm in self.in_names
        ]

    def run(self, in_maps):
        out_arrs = self.run_raw(self.concat_inputs(in_maps))
        n = self.n_cores
        return [
            {nm: np.asarray(out_arrs[i]).reshape(n, *self.out_avals[i].shape)[c]
             for i, nm in enumerate(self.out_names)}
            for c in range(n)
        ]


def get_runner():
    if "runner" not in _NC_CACHE:
        _NC_CACHE["runner"] = Runner(get_program())
    return _NC_CACHE["runner"]


def run(in_maps, **kw):
    from concourse.bass_utils import run_bass_kernel_spmd
    nc = get_program()
    return run_bass_kernel_spmd(nc, in_maps, core_ids=list(range(8)), **kw)


def kernel(features, radiance, global_features, params):
    in_maps = make_inputs(features, radiance, global_features, params)
    results = get_runner().run(in_maps)
    out = np.zeros((2, 3, 136, 136), np.float32)
    for b in range(2):
        for k in range(NB):
            out[b, :, OB * k:OB * (k + 1), :] = results[b * NB + k]["out"]
    return out


if __name__ == "__main__":
    import json
    nc = build_program()
    print("built OK, instructions:",
          sum(len(blk.instructions) for blk in []) or "n/a")
